# revision 1
# baseline (speedup 1.0000x reference)
"""GCN decoder (2-layer GCNConv + tanh) for Trainium2.

Self-contained: accepts FULL inputs, returns FULL output.
Strategy (per sharding hint): shard node rows across the 8 NeuronCores,
replicate weights; edges partitioned by destination-node shard so the
scatter-add is local to each shard; source features are all-gathered
(full x is visible to every shard) per layer.

Implementation: the whole GCN is expressed in JAX and executed on the
Neuron cores through PJRT with shard_map over an 8-core mesh. A pure
numpy fallback guarantees a correct answer if the device path fails.
"""
import numpy as np

N_NODES = 50000
N_CORES = 8


def _gcn_numpy(x, W, b, src_f, dst_f, norm):
    xw = x @ W
    msg = norm[:, None] * xw[src_f]
    out = np.zeros((N_NODES, W.shape[1]), dtype=np.float32)
    np.add.at(out, dst_f, msg)
    return out + b


def _prep(edge_index):
    src = edge_index[0].astype(np.int64)
    dst = edge_index[1].astype(np.int64)
    loop = np.arange(N_NODES, dtype=np.int64)
    src_f = np.concatenate([src, loop])
    dst_f = np.concatenate([dst, loop])
    deg = np.bincount(dst_f, minlength=N_NODES).astype(np.float32)
    d_inv_sqrt = np.where(deg > 0, 1.0 / np.sqrt(np.maximum(deg, 1e-12)), 0.0).astype(
        np.float32
    )
    norm = (d_inv_sqrt[src_f] * d_inv_sqrt[dst_f]).astype(np.float32)
    return src_f, dst_f, norm


def _kernel_jax(x, edge_index, W1, b1, W2, b2):
    import jax
    import jax.numpy as jnp
    from jax.sharding import Mesh, PartitionSpec as P
    from jax.experimental.shard_map import shard_map

    devs = jax.devices()[:N_CORES]
    mesh = Mesh(np.array(devs), ("i",))

    src_f, dst_f, norm = _prep(edge_index)
    E = src_f.shape[0]

    # Partition edges by destination shard so scatter-add is local.
    shard_size = N_NODES // N_CORES  # 6250
    owner = dst_f // shard_size
    order = np.argsort(owner, kind="stable")
    src_s = src_f[order]
    dst_s = dst_f[order]
    norm_s = norm[order]
    counts = np.bincount(owner, minlength=N_CORES)
    max_e = int(counts.max())
    # pad each shard's edge list to max_e with self-pointing zero-weight edges
    src_p = np.zeros((N_CORES, max_e), dtype=np.int32)
    dst_p = np.zeros((N_CORES, max_e), dtype=np.int32)
    nrm_p = np.zeros((N_CORES, max_e), dtype=np.float32)
    off = 0
    for c in range(N_CORES):
        n = counts[c]
        src_p[c, :n] = src_s[off : off + n]
        dst_p[c, :n] = dst_s[off : off + n] - c * shard_size  # local dst index
        nrm_p[c, :n] = norm_s[off : off + n]
        off += n

    def layer(x_full, W, b, src_l, dst_l, nrm_l):
        # x_full: [N, d_in] replicated; per-shard edge lists
        xw = x_full @ W  # replicated matmul (cheap: N x 128 x <=128)
        msg = nrm_l[:, None] * xw[src_l]
        out = jax.ops.segment_sum(msg, dst_l, num_segments=shard_size)
        return out + b  # [shard_size, d_out] local rows

    def fwd(x_full, W1_, b1_, W2_, b2_, src_l, dst_l, nrm_l):
        src_l, dst_l, nrm_l = src_l[0], dst_l[0], nrm_l[0]
        h_loc = layer(x_full, W1_, b1_, src_l, dst_l, nrm_l)  # [shard, d_h]
        h_full = jax.lax.all_gather(h_loc, "i", axis=0, tiled=True)  # [N, d_h]
        o_loc = layer(h_full, W2_, b2_, src_l, dst_l, nrm_l)
        return jnp.tanh(o_loc)

    fwd_sm = shard_map(
        fwd,
        mesh=mesh,
        in_specs=(P(), P(), P(), P(), P(), P("i"), P("i"), P("i")),
        out_specs=P("i"),
        check_rep=False,
    )
    fn = jax.jit(fwd_sm)
    out = fn(
        jnp.asarray(x),
        jnp.asarray(W1),
        jnp.asarray(b1),
        jnp.asarray(W2),
        jnp.asarray(b2),
        jnp.asarray(src_p),
        jnp.asarray(dst_p),
        jnp.asarray(nrm_p),
    )
    return np.asarray(out).astype(np.float32)


def kernel(x, edge_index, W1, b1, W2, b2):
    x = np.asarray(x, dtype=np.float32)
    edge_index = np.asarray(edge_index)
    W1 = np.asarray(W1, dtype=np.float32)
    b1 = np.asarray(b1, dtype=np.float32)
    W2 = np.asarray(W2, dtype=np.float32)
    b2 = np.asarray(b2, dtype=np.float32)
    try:
        return _kernel_jax(x, edge_index, W1, b1, W2, b2)
    except Exception:
        src_f, dst_f, norm = _prep(edge_index)
        h = _gcn_numpy(x, W1, b1, src_f, dst_f, norm)
        o = _gcn_numpy(h, W2, b2, src_f, dst_f, norm)
        return np.tanh(o).astype(np.float32)



# revision 3
# speedup vs baseline: 21.8443x; 21.8443x over previous
"""2-layer GCN (PyG GCNConv semantics + tanh) on 8 Trainium2 NeuronCores via Bass.

Strategy (per sharding hint): nodes sharded across 8 cores, weights replicated,
edges partitioned by destination shard so aggregation is local; source features
for layer 2 are all-gathered (layer 1 gathers from the replicated input).

Math: with dinv = (deg incl. self-loop)^-1/2, a GCN layer is
    out[d] = dinv[d] * ((sum_{e: s->d} dinv[s]*in[s] + dinv[d]*in[d]) @ W) + b
Aggregation commutes with the weight matmul, so the device gathers pre-scaled
rows g[n] = dinv[n]*in[n], segment-sums them per destination (feature-major,
fixed-length windows via degree-sorted node renumbering), then applies W with
the weight matrix stationary on the TensorEngine.

Device pipeline per core:
  layer 1: dma_gather(xp halves) -> segmented tensor_reduce -> @W1 ->
           PE-transpose -> *dinv^2 -> g2 shard (bf16) -> AllGather
  layer 2: dma_gather(g2 halves) -> segmented tensor_reduce -> @W2 ->
           PE-transpose -> tanh(dinv * .) -> output shard

Biases are handled exactly: b1/b2 are zero in this model family; a numpy
fallback covers nonzero biases (and any device failure).
"""

import numpy as np
import ml_dtypes

N_NODES = 50000
N_EDGES = 640000
D_IN = 128
D_H = 128
D_OUT = 64
CORES = 8
SHARD = N_NODES // CORES  # 6250
CHUNK_W = 512  # windows per chunk (= max matmul moving free dim)

BF16 = ml_dtypes.bfloat16


# ---------------------------------------------------------------- host prep

def _roundup(x, m):
    return (x + m - 1) // m * m


def _prep(edge_index):
    """Vectorized host-side graph preprocessing. Returns everything the
    device program and the pre/post permutations need."""
    src = edge_index[0].astype(np.int64)
    dst = edge_index[1].astype(np.int64)
    loop = np.arange(N_NODES, dtype=np.int64)
    src_f = np.concatenate([src, loop])
    dst_f = np.concatenate([dst, loop])

    deg = np.bincount(dst_f, minlength=N_NODES).astype(np.float64)
    dinv = (1.0 / np.sqrt(deg)).astype(np.float32)  # deg >= 1 (self loop)

    halfbit = src_f >= (N_NODES // 2)
    degA = np.bincount(dst_f[~halfbit], minlength=N_NODES)
    degB = np.bincount(dst_f[halfbit], minlength=N_NODES)
    L_node = np.maximum(degA, degB)  # window length per node, >= 1

    Lmax = int(L_node.max())
    counts = np.zeros((CORES, Lmax + 1), np.int64)
    for c in range(CORES):
        counts[c] = np.bincount(
            L_node[c * SHARD : (c + 1) * SHARD], minlength=Lmax + 1
        )
    NW_L = counts.max(axis=0)  # uniform per-class window counts
    base_total = int(NW_L.sum())
    NWT = _roundup(base_total + 64, CHUNK_W)
    NW_L[1] += NWT - base_total  # spare dummies live in class L=1
    HALF_ROWS = 4 * NWT
    assert HALF_ROWS <= 32768, HALF_ROWS

    class_start = np.zeros(Lmax + 2, np.int64)
    class_start[1:] = np.cumsum(NW_L)

    # per-window class length (same for every core)
    Lw = np.repeat(np.arange(Lmax + 1), NW_L)
    assert len(Lw) == NWT
    slotbase = np.zeros(NWT + 1, np.int64)
    slotbase[1:] = np.cumsum(Lw)
    S_half = int(slotbase[-1])

    # per-core window permutation: sort shard nodes by L (stable)
    perm = np.full((CORES, NWT), -1, np.int64)
    w_of_node = np.empty(N_NODES, np.int64)
    for c in range(CORES):
        ids = np.arange(c * SHARD, (c + 1) * SHARD)
        order = np.argsort(L_node[ids], kind="stable")
        Lvals = L_node[ids][order]
        runstart = np.searchsorted(Lvals, Lvals)
        posw = class_start[Lvals] + (np.arange(SHARD) - runstart)
        perm[c, posw] = ids[order]
        w_of_node[ids[order]] = posw
    npad = (np.arange(N_NODES) // SHARD) * NWT + w_of_node  # padded global id

    # zero rows (first class-1 dummy window of core 0 / core 4)
    zrowA = int(class_start[1] + counts[0, 1])
    zrowB = int(class_start[1] + counts[4, 1])
    assert perm[0, zrowA] == -1 and perm[4, zrowB] == -1

    # chunk layout (uniform across cores)
    n_chunks = NWT // CHUNK_W
    chunk_meta = []  # per chunk: (pslots, [(L, nwin, slot_off, col_off), ...])
    for k in range(n_chunks):
        w0, w1 = k * CHUNK_W, (k + 1) * CHUNK_W
        s0 = int(slotbase[w0])
        cnt = int(slotbase[w1] - s0)
        pslots = _roundup(max(cnt, 128), 128)
        segs = []
        w = w0
        while w < w1:
            Lc = int(Lw[w])
            wend = min(w1, int(class_start[Lc + 1]))
            segs.append(
                (Lc, wend - w, int(slotbase[w] - s0), w - w0)
            )
            w = wend
        chunk_meta.append((pslots, segs))

    # per-core slot index arrays
    slotsA = np.full((CORES, S_half), zrowA, np.int64)
    slotsB = np.full((CORES, S_half), zrowB, np.int64)
    core_of = dst_f // SHARD
    for c in range(CORES):
        m = core_of == c
        es, ed, eh = src_f[m], dst_f[m], halfbit[m]
        we = w_of_node[ed]
        key = eh.astype(np.int64) * NWT + we
        ordr = np.argsort(key, kind="stable")
        ks = key[ordr]
        newgrp = np.ones(len(ks), bool)
        newgrp[1:] = ks[1:] != ks[:-1]
        first = np.maximum.accumulate(np.where(newgrp, np.arange(len(ks)), 0))
        pos = np.arange(len(ks)) - first
        slot = slotbase[we[ordr]] + pos
        val = npad[es[ordr]]
        hh = eh[ordr]
        slotsA[c, slot[~hh]] = val[~hh]
        slotsB[c, slot[hh]] = val[hh] - HALF_ROWS

    # idx tile per core: per chunk, half A then half B, each wrap-16 padded
    def wrap16(flat):
        n = len(flat)
        a = flat.reshape(n // 16, 16).T.astype(np.int16)  # [16, n/16]
        return np.tile(a, (8, 1))

    idx_tiles = []
    for c in range(CORES):
        parts = []
        for k, (pslots, _segs) in enumerate(chunk_meta):
            s0 = int(slotbase[k * CHUNK_W])
            s1 = int(slotbase[(k + 1) * CHUNK_W])
            for sl, zr in ((slotsA[c], zrowA), (slotsB[c], zrowB)):
                seg = sl[s0:s1]
                if len(seg) < pslots:
                    seg = np.concatenate(
                        [seg, np.full(pslots - len(seg), zr, np.int64)]
                    )
                parts.append(wrap16(seg))
        idx_tiles.append(np.concatenate(parts, axis=1))

    return dict(
        dinv=dinv,
        perm=perm,
        npad=npad,
        NWT=NWT,
        HALF_ROWS=HALF_ROWS,
        chunk_meta=chunk_meta,
        idx_tiles=idx_tiles,
    )


def _blk(v, NWT):
    """[NWT] -> [128, NWT/128] with v[j*128+p] at [p, j]."""
    return np.ascontiguousarray(v.reshape(NWT // 128, 128).T)


# ---------------------------------------------------------------- device

def _build_bass(NWT, HALF_ROWS, chunk_meta, icols):
    from concourse import bass, tile, mybir

    BF = mybir.dt.bfloat16
    F32 = mybir.dt.float32
    I16 = mybir.dt.int16
    n_chunks = NWT // CHUNK_W

    nc = bass.Bass(
        "TRN2", target_bir_lowering=False, debug=False, num_devices=CORES
    )
    xp_ap = nc.dram_tensor(
        "xp", [CORES * NWT, D_IN], BF, kind="ExternalInput"
    ).ap()
    idx_ap = nc.dram_tensor("idx", [128, icols], I16, kind="ExternalInput").ap()
    w1_ap = nc.dram_tensor("w1", [D_IN, D_H], BF, kind="ExternalInput").ap()
    w2_ap = nc.dram_tensor("w2", [D_H, D_OUT], BF, kind="ExternalInput").ap()
    d2_ap = nc.dram_tensor(
        "dinv2", [128, NWT // 128], F32, kind="ExternalInput"
    ).ap()
    d1_ap = nc.dram_tensor(
        "dinv1", [128, NWT // 128], F32, kind="ExternalInput"
    ).ap()
    id_ap = nc.dram_tensor("ident", [128, 128], F32, kind="ExternalInput").ap()
    out_ap = nc.dram_tensor(
        "out", [NWT, D_OUT], F32, kind="ExternalOutput"
    ).ap()

    with tile.TileContext(nc) as tc:
        with (
            tc.tile_pool(name="const", bufs=1) as cpool,
            tc.tile_pool(name="msg", bufs=3) as mpool,
            tc.tile_pool(name="acc", bufs=3) as apool,
            tc.tile_pool(name="accb", bufs=3) as abpool,
            tc.tile_pool(name="psmm", bufs=2, space="PSUM") as psmm,
            tc.tile_pool(name="pstr", bufs=2, space="PSUM") as pstr,
            tc.tile_pool(name="sub", bufs=3) as spool,
            tc.tile_pool(name="gout", bufs=3) as gpool,
            tc.tile_pool(name="dram", bufs=1, space="DRAM") as dpool,
        ):
            w1_t = cpool.tile([D_IN, D_H], BF)
            nc.sync.dma_start(w1_t[:], w1_ap[:])
            w2_t = cpool.tile([D_H, D_OUT], BF)
            nc.sync.dma_start(w2_t[:], w2_ap[:])
            ident = cpool.tile([128, 128], F32)
            nc.sync.dma_start(ident[:], id_ap[:])
            d2_t = cpool.tile([128, NWT // 128], F32)
            nc.sync.dma_start(d2_t[:], d2_ap[:])
            d1_t = cpool.tile([128, NWT // 128], F32)
            nc.sync.dma_start(d1_t[:], d1_ap[:])
            idx_t = cpool.tile([128, icols], I16)
            nc.sync.dma_start(idx_t[:], idx_ap[:])

            g2_shard = dpool.tile([NWT, D_H], BF)
            g2_full = dpool.tile([CORES * NWT, D_H], BF)

            icol = [0]

            def layer(src_full, w_t, d_free, is_last):
                d_out = D_OUT if is_last else D_H
                for k in range(n_chunks):
                    pslots, segs = chunk_meta[k]
                    msg = mpool.tile([128, 2, pslots], BF, tag="msg")
                    for h in range(2):
                        nc.gpsimd.dma_gather(
                            msg[:, h : h + 1, :],
                            src_full[h * HALF_ROWS : (h + 1) * HALF_ROWS, :],
                            idx_t[:, icol[0] : icol[0] + pslots // 16],
                            pslots,
                            pslots,
                            d_free,
                        )
                        icol[0] += pslots // 16
                    acc = apool.tile([128, CHUNK_W], F32, tag="acc")
                    for (Lc, nwin, soff, coff) in segs:
                        view = msg[:, :, soff : soff + nwin * Lc].rearrange(
                            "p h (w l) -> p w h l", l=Lc
                        )
                        nc.vector.tensor_reduce(
                            acc[:, coff : coff + nwin],
                            view,
                            mybir.AxisListType.XY,
                            mybir.AluOpType.add,
                        )
                    accb = abpool.tile([128, CHUNK_W], BF, tag="accb")
                    nc.vector.tensor_copy(accb[:], acc[:])
                    ps = psmm.tile([d_out, CHUNK_W], F32, tag="psmm")
                    nc.tensor.matmul(
                        ps[:], w_t[:], accb[:], start=True, stop=True
                    )
                    for j in range(CHUNK_W // 128):
                        col = k * (CHUNK_W // 128) + j
                        sub = spool.tile([d_out, 128], F32, tag="sub")
                        nc.vector.tensor_copy(
                            sub[:], ps[:, j * 128 : (j + 1) * 128]
                        )
                        pst = pstr.tile([128, d_out], F32, tag="pstr")
                        nc.tensor.transpose(
                            pst[:], sub[:], ident[:d_out, :d_out]
                        )
                        if is_last:
                            ot = gpool.tile([128, D_OUT], F32, tag="got")
                            nc.scalar.activation(
                                ot[:],
                                pst[:],
                                mybir.ActivationFunctionType.Tanh,
                                scale=d1_t[:, col : col + 1],
                            )
                            nc.sync.dma_start(
                                out_ap[col * 128 : (col + 1) * 128, :], ot[:]
                            )
                        else:
                            gt = gpool.tile([128, D_H], BF, tag="g2t")
                            nc.vector.tensor_scalar(
                                gt[:],
                                pst[:],
                                d2_t[:, col : col + 1],
                                None,
                                mybir.AluOpType.mult,
                            )
                            nc.sync.dma_start(
                                g2_shard[col * 128 : (col + 1) * 128, :], gt[:]
                            )

            layer(xp_ap, w1_t, D_IN, is_last=False)
            nc.gpsimd.collective_compute(
                "AllGather",
                mybir.AluOpType.bypass,
                replica_groups=[list(range(CORES))],
                ins=[g2_shard.opt()],
                outs=[g2_full.opt()],
            )
            layer(g2_full, w2_t, D_H, is_last=True)

    return nc


# ---------------------------------------------------------------- fallback

def _gcn_numpy(x, W, b, src_f, dst_f, norm):
    xw = x @ W
    msg = norm[:, None] * xw[src_f]
    out = np.zeros((N_NODES, W.shape[1]), dtype=np.float32)
    np.add.at(out, dst_f, msg)
    return out + b


def _fallback(x, edge_index, W1, b1, W2, b2):
    src = edge_index[0].astype(np.int64)
    dst = edge_index[1].astype(np.int64)
    loop = np.arange(N_NODES, dtype=np.int64)
    src_f = np.concatenate([src, loop])
    dst_f = np.concatenate([dst, loop])
    deg = np.bincount(dst_f, minlength=N_NODES).astype(np.float32)
    dinv = np.where(deg > 0, 1.0 / np.sqrt(np.maximum(deg, 1e-12)), 0.0)
    norm = (dinv[src_f] * dinv[dst_f]).astype(np.float32)
    h = _gcn_numpy(x, W1, b1, src_f, dst_f, norm)
    o = _gcn_numpy(h, W2, b2, src_f, dst_f, norm)
    return np.tanh(o).astype(np.float32)


# ---------------------------------------------------------------- entry

_LAST_RESULTS = {}


def kernel(x, edge_index, W1, b1, W2, b2):
    x = np.asarray(x, dtype=np.float32)
    edge_index = np.asarray(edge_index)
    W1 = np.asarray(W1, dtype=np.float32)
    b1 = np.asarray(b1, dtype=np.float32)
    W2 = np.asarray(W2, dtype=np.float32)
    b2 = np.asarray(b2, dtype=np.float32)
    if np.any(b1) or np.any(b2):
        return _fallback(x, edge_index, W1, b1, W2, b2)
    try:
        return _kernel_device(x, edge_index, W1, W2)
    except Exception:
        import traceback

        traceback.print_exc()
        return _fallback(x, edge_index, W1, b1, W2, b2)


def _kernel_device(x, edge_index, W1, W2):
    from concourse import bass_utils

    p = _prep(edge_index)
    NWT = p["NWT"]
    dinv, perm, npad = p["dinv"], p["perm"], p["npad"]

    # padded, dinv-scaled, permuted source features (zero rows for dummies)
    xs = (dinv[:, None] * x).astype(BF16)
    xp = np.zeros((CORES * NWT, D_IN), BF16)
    xp[npad] = xs

    icols = p["idx_tiles"][0].shape[1]
    nc = _build_bass(NWT, p["HALF_ROWS"], p["chunk_meta"], icols)

    d2 = np.zeros((CORES, NWT), np.float32)
    d1 = np.zeros((CORES, NWT), np.float32)
    for c in range(CORES):
        m = perm[c] >= 0
        d2[c, m] = dinv[perm[c, m]] ** 2
        d1[c, m] = dinv[perm[c, m]]

    w1b = W1.astype(BF16)
    w2b = W2.astype(BF16)
    eye = np.eye(128, dtype=np.float32)
    in_maps = []
    for c in range(CORES):
        in_maps.append(
            {
                "xp": xp,
                "idx": p["idx_tiles"][c],
                "w1": w1b,
                "w2": w2b,
                "dinv2": _blk(d2[c], NWT),
                "dinv1": _blk(d1[c], NWT),
                "ident": eye,
            }
        )

    import os

    res = bass_utils.run_bass_kernel_spmd(
        nc,
        in_maps,
        core_ids=list(range(CORES)),
        trace=bool(int(os.environ.get("GCN_TRACE", "0"))),
    )
    _LAST_RESULTS["res"] = res

    out = np.empty((N_NODES, D_OUT), np.float32)
    for c in range(CORES):
        m = perm[c] >= 0
        out[perm[c, m]] = res.results[c]["out"][m]
    return out
